# revision 38
# baseline (speedup 1.0000x reference)
"""Binomial-deviance loss (cosine-similarity based) on 8 Trainium2 cores.

v3: bf16 inputs + transposed layout + PE-matmul reductions, pipelined.

The 2e-2 rel-err budget is ~4 orders of magnitude above what fp32 gives, so
inputs are downcast to bf16 on the host (halves HBM traffic: 67MB -> 33.5MB
per core, DMA floor ~94us at ~360GB/s/core). The host also pre-transposes
each core slice to d-major [512, 16384] so the per-row reductions over D=512
become partition-axis reductions, which the Tensor engine does via
ones-vector matmuls -- freeing the DVE from its 1x-only tensor_reduce.

Per core (row tiles: 12x1024 then a tapered 8x512 tail round; 4 d-chunks of
128 partitions each):
  DVE: prod = o1*o2 (bf16 TT 2x) + sq2 = o2*o2
  ACT: sq1 = o1*o1 (Square, dtype-independent 1x)
  PE : dot/n1/n2 = ones[128,32]^T @ {prod,sq1,sq2} -> [32,512] PSUM stripes
       (32 replicated rows; M=32 because matmul output base partition must
       be 0/32/64 and engine APs reject partition strides, so replication
       makes the drain a contiguous [0:96] read): 512-row block B ->
       bank B%8, partitions [32t, 32t+32), accumulated over the 4 d-chunks
  ACT: drain per 2048-row half-round h: copy psum[0:96, (h%2)*2048:+2048]
       -> SBUF stage [96,2048] (PSUM is not DMA-able in this stack); banks
       ping-pong in halves of 4
  DMA: scatter stage -> acc[128, 3*128] in natural row order (row r of the
       core lands at partition r//128, col r%128); dma matches flat element
       order so a contiguous [1,2048] src feeds a [16,128] dst directly
Tail on [128,128] acc slices, computed in two partition-halves (first half
mid-kernel once its acc rows have landed, second at the end) to shorten the
serial endgame: d = dot*exp(-0.5*ln(n1*n2)), softplus via ln(1+exp(x)),
masked sums -> [128,3] partials; host reduces 8x128x3 and divides.

This walrus build only accepts ONE semaphore wait per instruction, while Tile
emits multi-wait sync_info; a post-pass hoists overflow waits onto injected
same-engine InstNoOps.
"""

import sys

import numpy as np

if "/opt/trn_rl_repo" not in sys.path:
    try:
        import concourse  # noqa: F401
    except ImportError:
        sys.path.insert(0, "/opt/trn_rl_repo")

N, D = 131072, 512
NCORES = 8
CORE_ROWS = N // NCORES  # 16384
P = 128  # partitions
NCHUNK = D // P  # 4 d-chunks
ALPHA = 50.0
BETA = 0.5

# row tiles: 7 x 2048 rows (4KB DMA runs), then 4 x 512 (tapered endgame)
TILES = [(i * 2048, 2048) for i in range(7)] + [
    (14336 + i * 512, 512) for i in range(4)
]

_CACHE = {}


def _split_waits(nc, mybir, maxw=1):
    """walrus here rejects >1 sync wait per instruction; hoist extras onto
    injected same-engine NoOps placed immediately before the instruction."""
    for fn in nc.m.functions:
        for blk in fn.blocks:
            new_insts = []
            for inst in blk.instructions:
                si = inst.sync_info
                if si is not None and si.on_wait and len(si.on_wait) > maxw:
                    waits = list(si.on_wait)
                    k = 0
                    while len(waits) - k > maxw:
                        chunk = waits[k : k + maxw]
                        k += maxw
                        nop = mybir.InstNoOp(
                            name=f"{inst.name}-ws{k}", ins=[], outs=[]
                        )
                        nop.engine = inst.engine
                        nop.sync_info = mybir.SyncInfo(on_wait=chunk, on_update=[])
                        new_insts.append(nop)
                    inst.sync_info = mybir.SyncInfo(
                        on_wait=waits[k:], on_update=list(si.on_update or [])
                    )
                new_insts.append(inst)
            blk.instructions = new_insts


def _build_nc():
    import concourse.bass as bass
    import concourse.mybir as mybir
    from concourse.tile import TileContext

    fp32 = mybir.dt.float32
    bf16 = mybir.dt.bfloat16
    Act = mybir.ActivationFunctionType
    Alu = mybir.AluOpType

    nc = bass.Bass()
    o1 = nc.dram_tensor("o1", [D, CORE_ROWS], bf16, kind="ExternalInput")
    o2 = nc.dram_tensor("o2", [D, CORE_ROWS], bf16, kind="ExternalInput")
    mask = nc.dram_tensor("mask", [P, P], fp32, kind="ExternalInput")
    out = nc.dram_tensor("partials", [P, 2], fp32, kind="ExternalOutput")

    with TileContext(nc) as tc:
        with (
            tc.tile_pool(name="data", bufs=2) as dpool,
            tc.tile_pool(name="work", bufs=2) as wpool,
            tc.tile_pool(name="stg", bufs=2) as spool,
            tc.tile_pool(name="acc", bufs=1) as apool,
            tc.tile_pool(name="psum", bufs=1, space="PSUM") as ppool,
        ):
            mask_t = apool.tile([P, P], fp32, tag="mask_t")
            ones_t = apool.tile([P, 32], bf16, tag="ones_t")
            acc_t = apool.tile([P, 3 * P], fp32, tag="acc_t")
            b_pos = apool.tile([P, 1], fp32, tag="b_pos")

            nc.gpsimd.memset(ones_t[:, :], 1.0)
            nc.gpsimd.memset(b_pos[:, :], BETA / 2.0)

            # tail tiles (partition-sliced for the two-half tail)
            nn_t = apool.tile([P, P], fp32, tag="nn_t")
            rs_t = apool.tile([P, P], fp32, tag="rs_t")
            d_t = apool.tile([P, P], fp32, tag="d_t")
            e_t = apool.tile([P, P], fp32, tag="e_t")
            sp_t = apool.tile([P, P], fp32, tag="sp_t")
            f_t = apool.tile([P, P], fp32, tag="f_t")
            out_t = apool.tile([P, 2], fp32, tag="out_t")
            one = nc.const_aps.scalar_like(1.0, nn_t[:, :])

            def tail_half(pl, ph):
                # neg branch dropped: d = cosine sim <= 1, so
                # (2/A)*softplus(A*(d-2)) <= 0.04*e^-50 ~ 8e-24 per element --
                # identically zero at fp32 scale for ANY input.
                sl = slice(pl, ph)
                dot_a = acc_t[sl, 0:P]
                n1_a = acc_t[sl, P : 2 * P]
                n2_a = acc_t[sl, 2 * P : 3 * P]
                nc.vector.tensor_mul(out=nn_t[sl, :], in0=n1_a, in1=n2_a)
                # 1/sqrt(nn) = exp(-0.5*ln(nn)); ln/exp share one table set
                nc.scalar.activation(out=rs_t[sl, :], in_=nn_t[sl, :], func=Act.Ln)
                nc.scalar.activation(
                    out=rs_t[sl, :], in_=rs_t[sl, :], func=Act.Exp, scale=-0.5
                )
                nc.vector.tensor_mul(out=d_t[sl, :], in0=dot_a, in1=rs_t[sl, :])
                # pos = (2/B)*softplus(-B*d + B/2) = (2/B)*ln(1+exp(-B*d+B/2))
                nc.scalar.activation(
                    out=e_t[sl, :], in_=d_t[sl, :], func=Act.Exp,
                    bias=b_pos[sl, :], scale=-BETA,
                )
                nc.scalar.activation(
                    out=sp_t[sl, :], in_=e_t[sl, :], func=Act.Ln, bias=one[sl, :]
                )
                nc.vector.tensor_mul(
                    out=f_t[sl, :], in0=sp_t[sl, :], in1=mask_t[sl, :]
                )
                nc.vector.tensor_reduce(
                    out=out_t[sl, 0:1], in_=f_t[sl, :],
                    axis=mybir.AxisListType.X, op=Alu.add,
                )
                nc.vector.tensor_reduce(
                    out=out_t[sl, 1:2], in_=mask_t[sl, :],
                    axis=mybir.AxisListType.X, op=Alu.add,
                )

            # dram views: [partition(d%128), chunk(d//128), row]
            o1v = o1[:, :].rearrange("(c p) r -> p c r", c=NCHUNK, p=P)
            o2v = o2[:, :].rearrange("(c p) r -> p c r", c=NCHUNK, p=P)

            # all 8 PSUM banks: bank = 512-row block index % 8,
            # partition offset 32*t = target (dot/n1/n2)
            ps_t = ppool.tile([P, 8 * 512], fp32, tag="ps")
            for row0, nrows in TILES:
                W = NCHUNK * nrows
                t1f = dpool.tile([P, NCHUNK * 2048], bf16, tag="t1")
                t2f = dpool.tile([P, NCHUNK * 2048], bf16, tag="t2")
                prodf = wpool.tile([P, NCHUNK * 2048], bf16, tag="pr")
                sq1f = wpool.tile([P, NCHUNK * 2048], bf16, tag="s1")
                sq2f = wpool.tile([P, NCHUNK * 2048], bf16, tag="s2")
                t1, t2 = t1f[:, :W], t2f[:, :W]
                prod, sq1, sq2 = prodf[:, :W], sq1f[:, :W], sq2f[:, :W]
                nc.sync.dma_start(
                    out=t1[:, :].rearrange("p (c r) -> p c r", c=NCHUNK),
                    in_=o1v[:, :, row0 : row0 + nrows],
                )
                nc.sync.dma_start(
                    out=t2[:, :].rearrange("p (c r) -> p c r", c=NCHUNK),
                    in_=o2v[:, :, row0 : row0 + nrows],
                )
                if row0 == 0:
                    # after the first tile loads so it doesn't delay the ramp
                    nc.sync.dma_start(out=mask_t[:, :], in_=mask[:, :])

                nc.vector.tensor_mul(out=prod[:, :], in0=t1[:, :], in1=t2[:, :])
                nc.scalar.activation(out=sq1[:, :], in_=t1[:, :], func=Act.Square)
                nc.vector.tensor_mul(out=sq2[:, :], in0=t2[:, :], in1=t2[:, :])

                for ti, src in enumerate((prod, sq1, sq2)):
                    for j in range(nrows // 512):
                        q = (row0 // 512 + j) % 8
                        for c in range(NCHUNK):
                            nc.tensor.matmul(
                                out=ps_t[32 * ti : 32 * ti + 32, q * 512 : (q + 1) * 512],
                                lhsT=ones_t[:, :],
                                rhs=src[:, c * nrows + j * 512 : c * nrows + j * 512 + 512],
                                start=(c == 0),
                                stop=(c == NCHUNK - 1),
                            )

                rend = row0 + nrows
                if rend % 2048 == 0:
                    hr = rend // 2048 - 1  # half-round just completed
                    h = hr % 2
                    stage = spool.tile([96, 2048], fp32, tag="stage")
                    nc.scalar.copy(
                        stage[:, :], ps_t[0:96, h * 2048 : (h + 1) * 2048]
                    )
                    # scatter to natural row order: row r -> acc[r//128, r%128]
                    # via the idle gpsimd software DGE: keeps these
                    # drain-gated DMAs out of the sync engine's in-order
                    # issue queue, which they were head-of-line blocking.
                    # Last half-round: fan out across three DGEs so the
                    # endgame chain isn't serialized on one queue.
                    if hr == 7:
                        dges = (nc.gpsimd, nc.sync, nc.scalar)
                    else:
                        dges = (nc.gpsimd, nc.gpsimd, nc.gpsimd)
                    for ti in range(3):
                        dges[ti].dma_start(
                            out=acc_t[hr * 16 : (hr + 1) * 16, ti * P : (ti + 1) * P],
                            in_=stage[32 * ti : 32 * ti + 1, :],
                        )
                    if hr == 3:
                        tail_half(0, 64)  # acc rows 0..8191 have landed

            tail_half(64, P)
            nc.sync.dma_start(out=out[:, :], in_=out_t[:, :])

    _split_waits(nc, mybir, maxw=1)
    return nc


def _get_nc():
    if "nc" not in _CACHE:
        _CACHE["nc"] = _build_nc()
    return _CACHE["nc"]


def _make_in_maps(output1, output2, target):
    import ml_dtypes

    bf = ml_dtypes.bfloat16
    o1 = np.asarray(output1, dtype=np.float32).astype(bf)
    o2 = np.asarray(output2, dtype=np.float32).astype(bf)
    mask_full = (np.asarray(target) == 1).astype(np.float32)
    in_maps = []
    for c in range(NCORES):
        sl = slice(c * CORE_ROWS, (c + 1) * CORE_ROWS)
        in_maps.append(
            {
                "o1": np.ascontiguousarray(o1[sl].T),
                "o2": np.ascontiguousarray(o2[sl].T),
                "mask": mask_full[sl].reshape(P, P),
            }
        )
    return in_maps


def _combine(results):
    parts = np.stack([r["partials"] for r in results]).astype(np.float64)
    pos_sum, num_pos = parts.sum(axis=(0, 1))
    num_pos = int(round(num_pos))
    # neg branch is identically 0 at fp32 scale (see tail_half comment)
    pos_loss = np.float32((2.0 / BETA) * pos_sum) / np.float32(max(num_pos, 1))
    return np.float32(pos_loss)


def _run(output1, output2, target, trace=False, **spmd_kwargs):
    from concourse.bass_utils import run_bass_kernel_spmd

    nc = _get_nc()
    in_maps = _make_in_maps(output1, output2, target)
    res = run_bass_kernel_spmd(
        nc, in_maps, core_ids=list(range(NCORES)), trace=trace, **spmd_kwargs
    )
    return _combine(res.results), res


def kernel(output1, output2, target):
    try:
        loss, _ = _run(output1, output2, target, trace=False)
    except Exception:
        # transient NRT/device hiccups (e.g. NRT_EXEC_UNIT_UNRECOVERABLE)
        # usually clear on retry
        import time

        time.sleep(2.0)
        loss, _ = _run(output1, output2, target, trace=False)
    return loss


# revision 41
# speedup vs baseline: 1.0944x; 1.0944x over previous
"""Binomial-deviance loss (cosine-similarity based) on 8 Trainium2 cores.

v3: bf16 inputs + transposed layout + PE-matmul reductions, pipelined.

The 2e-2 rel-err budget is ~4 orders of magnitude above what fp32 gives, so
inputs are downcast to bf16 on the host (halves HBM traffic: 67MB -> 33.5MB
per core, DMA floor ~94us at ~360GB/s/core). The host also pre-transposes
each core slice to d-major [512, 16384] so the per-row reductions over D=512
become partition-axis reductions, which the Tensor engine does via
ones-vector matmuls -- freeing the DVE from its 1x-only tensor_reduce.

Per core (row tiles: 12x1024 then a tapered 8x512 tail round; 4 d-chunks of
128 partitions each):
  DVE: prod = o1*o2 (bf16 TT 2x) + sq2 = o2*o2
  ACT: sq1 = o1*o1 (Square, dtype-independent 1x)
  PE : dot/n1/n2 = ones[128,32]^T @ {prod,sq1,sq2} -> [32,512] PSUM stripes
       (32 replicated rows; M=32 because matmul output base partition must
       be 0/32/64 and engine APs reject partition strides, so replication
       makes the drain a contiguous [0:96] read): 512-row block B ->
       bank B%8, partitions [32t, 32t+32), accumulated over the 4 d-chunks
  ACT: drain per 2048-row half-round h: copy psum[0:96, (h%2)*2048:+2048]
       -> SBUF stage [96,2048] (PSUM is not DMA-able in this stack); banks
       ping-pong in halves of 4
  DMA: scatter stage -> acc[128, 3*128] in natural row order (row r of the
       core lands at partition r//128, col r%128); dma matches flat element
       order so a contiguous [1,2048] src feeds a [16,128] dst directly
Tail on [128,128] acc slices, computed in two partition-halves (first half
mid-kernel once its acc rows have landed, second at the end) to shorten the
serial endgame: d = dot*exp(-0.5*ln(n1*n2)), softplus via ln(1+exp(x)),
masked sums -> [128,3] partials; host reduces 8x128x3 and divides.

This walrus build only accepts ONE semaphore wait per instruction, while Tile
emits multi-wait sync_info; a post-pass hoists overflow waits onto injected
same-engine InstNoOps.
"""

import sys

import numpy as np

if "/opt/trn_rl_repo" not in sys.path:
    try:
        import concourse  # noqa: F401
    except ImportError:
        sys.path.insert(0, "/opt/trn_rl_repo")

N, D = 131072, 512
NCORES = 8
CORE_ROWS = N // NCORES  # 16384
P = 128  # partitions
NCHUNK = D // P  # 4 d-chunks
ALPHA = 50.0
BETA = 0.5

# row tiles: 14 x 1024 rows, then 4 x 512 (tapered endgame)
TILES = [(i * 1024, 1024) for i in range(14)] + [
    (14336 + i * 512, 512) for i in range(4)
]

_CACHE = {}


def _split_waits(nc, mybir, maxw=1):
    """walrus here rejects >1 sync wait per instruction; hoist extras onto
    injected same-engine NoOps placed immediately before the instruction."""
    for fn in nc.m.functions:
        for blk in fn.blocks:
            new_insts = []
            for inst in blk.instructions:
                si = inst.sync_info
                if si is not None and si.on_wait and len(si.on_wait) > maxw:
                    waits = list(si.on_wait)
                    k = 0
                    while len(waits) - k > maxw:
                        chunk = waits[k : k + maxw]
                        k += maxw
                        nop = mybir.InstNoOp(
                            name=f"{inst.name}-ws{k}", ins=[], outs=[]
                        )
                        nop.engine = inst.engine
                        nop.sync_info = mybir.SyncInfo(on_wait=chunk, on_update=[])
                        new_insts.append(nop)
                    inst.sync_info = mybir.SyncInfo(
                        on_wait=waits[k:], on_update=list(si.on_update or [])
                    )
                new_insts.append(inst)
            blk.instructions = new_insts


def _build_nc():
    import concourse.bass as bass
    import concourse.mybir as mybir
    from concourse.tile import TileContext

    fp32 = mybir.dt.float32
    bf16 = mybir.dt.bfloat16
    Act = mybir.ActivationFunctionType
    Alu = mybir.AluOpType

    nc = bass.Bass()
    o1 = nc.dram_tensor("o1", [D, CORE_ROWS], bf16, kind="ExternalInput")
    o2 = nc.dram_tensor("o2", [D, CORE_ROWS], bf16, kind="ExternalInput")
    mask = nc.dram_tensor("mask", [P, P], fp32, kind="ExternalInput")
    out = nc.dram_tensor("partials", [P, 2], fp32, kind="ExternalOutput")

    with TileContext(nc) as tc:
        with (
            tc.tile_pool(name="data", bufs=4) as dpool,
            tc.tile_pool(name="work", bufs=4) as wpool,
            tc.tile_pool(name="stg", bufs=2) as spool,
            tc.tile_pool(name="acc", bufs=1) as apool,
            tc.tile_pool(name="psum", bufs=1, space="PSUM") as ppool,
        ):
            mask_t = apool.tile([P, P], fp32, tag="mask_t")
            ones_t = apool.tile([P, 32], bf16, tag="ones_t")
            acc_t = apool.tile([P, 3 * P], fp32, tag="acc_t")
            b_pos = apool.tile([P, 1], fp32, tag="b_pos")

            nc.gpsimd.memset(ones_t[:, :], 1.0)
            nc.gpsimd.memset(b_pos[:, :], BETA / 2.0)

            # tail tiles (partition-sliced for the two-half tail)
            nn_t = apool.tile([P, P], fp32, tag="nn_t")
            rs_t = apool.tile([P, P], fp32, tag="rs_t")
            d_t = apool.tile([P, P], fp32, tag="d_t")
            e_t = apool.tile([P, P], fp32, tag="e_t")
            sp_t = apool.tile([P, P], fp32, tag="sp_t")
            f_t = apool.tile([P, P], fp32, tag="f_t")
            out_t = apool.tile([P, 2], fp32, tag="out_t")
            one = nc.const_aps.scalar_like(1.0, nn_t[:, :])

            def tail_half(pl, ph):
                # neg branch dropped: d = cosine sim <= 1, so
                # (2/A)*softplus(A*(d-2)) <= 0.04*e^-50 ~ 8e-24 per element --
                # identically zero at fp32 scale for ANY input.
                sl = slice(pl, ph)
                dot_a = acc_t[sl, 0:P]
                n1_a = acc_t[sl, P : 2 * P]
                n2_a = acc_t[sl, 2 * P : 3 * P]
                nc.vector.tensor_mul(out=nn_t[sl, :], in0=n1_a, in1=n2_a)
                # 1/sqrt(nn) = exp(-0.5*ln(nn)); ln/exp share one table set
                nc.scalar.activation(out=rs_t[sl, :], in_=nn_t[sl, :], func=Act.Ln)
                nc.scalar.activation(
                    out=rs_t[sl, :], in_=rs_t[sl, :], func=Act.Exp, scale=-0.5
                )
                nc.vector.tensor_mul(out=d_t[sl, :], in0=dot_a, in1=rs_t[sl, :])
                # pos = (2/B)*softplus(-B*d + B/2) = (2/B)*ln(1+exp(-B*d+B/2))
                nc.scalar.activation(
                    out=e_t[sl, :], in_=d_t[sl, :], func=Act.Exp,
                    bias=b_pos[sl, :], scale=-BETA,
                )
                nc.scalar.activation(
                    out=sp_t[sl, :], in_=e_t[sl, :], func=Act.Ln, bias=one[sl, :]
                )
                nc.vector.tensor_mul(
                    out=f_t[sl, :], in0=sp_t[sl, :], in1=mask_t[sl, :]
                )
                nc.vector.tensor_reduce(
                    out=out_t[sl, 0:1], in_=f_t[sl, :],
                    axis=mybir.AxisListType.X, op=Alu.add,
                )
                nc.vector.tensor_reduce(
                    out=out_t[sl, 1:2], in_=mask_t[sl, :],
                    axis=mybir.AxisListType.X, op=Alu.add,
                )

            # dram views: [partition(d%128), chunk(d//128), row]
            o1v = o1[:, :].rearrange("(c p) r -> p c r", c=NCHUNK, p=P)
            o2v = o2[:, :].rearrange("(c p) r -> p c r", c=NCHUNK, p=P)

            # all 8 PSUM banks: bank = 512-row block index % 8,
            # partition offset 32*t = target (dot/n1/n2)
            ps_t = ppool.tile([P, 8 * 512], fp32, tag="ps")
            for row0, nrows in TILES:
                t1 = dpool.tile([P, NCHUNK * nrows], bf16, tag=f"t1_{nrows}")
                t2 = dpool.tile([P, NCHUNK * nrows], bf16, tag=f"t2_{nrows}")
                prod = wpool.tile([P, NCHUNK * nrows], bf16, tag=f"pr_{nrows}")
                nc.sync.dma_start(
                    out=t1[:, :].rearrange("p (c r) -> p c r", c=NCHUNK),
                    in_=o1v[:, :, row0 : row0 + nrows],
                )
                nc.sync.dma_start(
                    out=t2[:, :].rearrange("p (c r) -> p c r", c=NCHUNK),
                    in_=o2v[:, :, row0 : row0 + nrows],
                )
                if row0 == 0:
                    # after the first tile loads so it doesn't delay the ramp
                    nc.sync.dma_start(out=mask_t[:, :], in_=mask[:, :])

                # squares in place (after prod) -- saves two SBUF rings,
                # buying bufs=4 prefetch depth
                nc.vector.tensor_mul(out=prod[:, :], in0=t1[:, :], in1=t2[:, :])
                nc.scalar.activation(out=t1[:, :], in_=t1[:, :], func=Act.Square)
                nc.vector.tensor_mul(out=t2[:, :], in0=t2[:, :], in1=t2[:, :])

                for ti, src in enumerate((prod, t1, t2)):
                    for j in range(nrows // 512):
                        q = (row0 // 512 + j) % 8
                        for c in range(NCHUNK):
                            nc.tensor.matmul(
                                out=ps_t[32 * ti : 32 * ti + 32, q * 512 : (q + 1) * 512],
                                lhsT=ones_t[:, :],
                                rhs=src[:, c * nrows + j * 512 : c * nrows + j * 512 + 512],
                                start=(c == 0),
                                stop=(c == NCHUNK - 1),
                            )

                rend = row0 + nrows
                if rend % 2048 == 0:
                    hr = rend // 2048 - 1  # half-round just completed
                    h = hr % 2
                    stage = spool.tile([96, 2048], fp32, tag="stage")
                    nc.scalar.copy(
                        stage[:, :], ps_t[0:96, h * 2048 : (h + 1) * 2048]
                    )
                    # scatter to natural row order: row r -> acc[r//128, r%128]
                    # via the idle gpsimd software DGE: keeps these
                    # drain-gated DMAs out of the sync engine's in-order
                    # issue queue, which they were head-of-line blocking.
                    # Last half-round: fan out across three DGEs so the
                    # endgame chain isn't serialized on one queue.
                    if hr == 7:
                        dges = (nc.gpsimd, nc.sync, nc.scalar)
                    else:
                        dges = (nc.gpsimd, nc.gpsimd, nc.gpsimd)
                    for ti in range(3):
                        dges[ti].dma_start(
                            out=acc_t[hr * 16 : (hr + 1) * 16, ti * P : (ti + 1) * P],
                            in_=stage[32 * ti : 32 * ti + 1, :],
                        )
                    if hr == 3:
                        tail_half(0, 64)  # acc rows 0..8191 have landed

            tail_half(64, P)
            nc.sync.dma_start(out=out[:, :], in_=out_t[:, :])

    _split_waits(nc, mybir, maxw=1)
    return nc


def _get_nc():
    if "nc" not in _CACHE:
        _CACHE["nc"] = _build_nc()
    return _CACHE["nc"]


def _make_in_maps(output1, output2, target):
    import ml_dtypes

    bf = ml_dtypes.bfloat16
    o1 = np.asarray(output1, dtype=np.float32).astype(bf)
    o2 = np.asarray(output2, dtype=np.float32).astype(bf)
    mask_full = (np.asarray(target) == 1).astype(np.float32)
    in_maps = []
    for c in range(NCORES):
        sl = slice(c * CORE_ROWS, (c + 1) * CORE_ROWS)
        in_maps.append(
            {
                "o1": np.ascontiguousarray(o1[sl].T),
                "o2": np.ascontiguousarray(o2[sl].T),
                "mask": mask_full[sl].reshape(P, P),
            }
        )
    return in_maps


def _combine(results):
    parts = np.stack([r["partials"] for r in results]).astype(np.float64)
    pos_sum, num_pos = parts.sum(axis=(0, 1))
    num_pos = int(round(num_pos))
    # neg branch is identically 0 at fp32 scale (see tail_half comment)
    pos_loss = np.float32((2.0 / BETA) * pos_sum) / np.float32(max(num_pos, 1))
    return np.float32(pos_loss)


def _run(output1, output2, target, trace=False, **spmd_kwargs):
    from concourse.bass_utils import run_bass_kernel_spmd

    nc = _get_nc()
    in_maps = _make_in_maps(output1, output2, target)
    res = run_bass_kernel_spmd(
        nc, in_maps, core_ids=list(range(NCORES)), trace=trace, **spmd_kwargs
    )
    return _combine(res.results), res


def kernel(output1, output2, target):
    try:
        loss, _ = _run(output1, output2, target, trace=False)
    except Exception:
        # transient NRT/device hiccups (e.g. NRT_EXEC_UNIT_UNRECOVERABLE)
        # usually clear on retry
        import time

        time.sleep(2.0)
        loss, _ = _run(output1, output2, target, trace=False)
    return loss
